# revision 22
# baseline (speedup 1.0000x reference)
"""Trainium2 Bass kernel for nn_AttentionLayer (B=8, N=1024, D=1024, H=16).

Sharding: data-parallel over batch -- one batch element per NeuronCore (8 cores).

Per-core plan (activations kept transposed [D, tokens] so every matmul maps
natively onto the PE array's partition-dim contraction):
  1. LayerNorm x / source in natural layout (bn_stats over free dim); gains and
     biases of the pre-norms are folded into the projection weights on the host.
  2. PE-transpose the normalized activations -> snT/xnT [D, tok].
  3. Projections: qT/kT = W.T @ snT|xnT (transposed out; bias + RoPE fused into
     the PSUM epilogue via stream_shuffle + scalar_tensor_tensor);
     v natural [tok, D] via xnT-stationary matmul, ones column packed per head.
  4. Attention per head: scoresT = kT-block stationary x qT moving; exp on ACT
     with fused 1/sqrt(dh) scale; attn@v with v|ones stationary yields both
     output^T and the softmax denominator in one PSUM accumulation chain.
  5. Normalize by the reciprocal denominator (DRAM-bounce partition broadcast),
     out-projection with attn_outT stationary producing the NATURAL-layout
     result, final LayerNorm in natural layout, store.
All matmul operands are float32r (TF32-class, full PE rate at N=512).
"""

import numpy as np

import concourse.bass as bass
import concourse.tile as tile
from concourse import bacc, mybir
from concourse.bass_utils import run_bass_kernel_spmd
from concourse.masks import make_identity

F32 = mybir.dt.float32
F32R = mybir.dt.float32r

B, N, D, H = 8, 1024, 1024, 16
DH = D // H  # 64
EPS = 1e-5
THETA = 10000.0
NT = N // 128  # 8 token tiles
DT = D // 128  # 8 channel tiles
SCALE = float(DH) ** -0.5

_PAIRSWAP = []
for _i in range(16):
    _PAIRSWAP += [2 * _i + 1, 2 * _i]


def build_program():
    nc = bacc.Bacc("TRN2", target_bir_lowering=False, debug=False)

    x_d = nc.dram_tensor("x", [N, D], F32, kind="ExternalInput").ap()
    s_d = nc.dram_tensor("src", [N, D], F32, kind="ExternalInput").ap()
    wq_d = nc.dram_tensor("wq", [D, D], F32R, kind="ExternalInput").ap()
    wk_d = nc.dram_tensor("wk", [D, D], F32R, kind="ExternalInput").ap()
    wv_d = nc.dram_tensor("wv", [D, D], F32R, kind="ExternalInput").ap()
    wo_d = nc.dram_tensor("wo", [D, D], F32R, kind="ExternalInput").ap()
    cosf_d = nc.dram_tensor("cosf", [128, N], F32, kind="ExternalInput").ap()
    sinf_d = nc.dram_tensor("sinf", [128, N], F32, kind="ExternalInput").ap()
    # bias tables [128, DT] (col t = do-tile t); *s = pair-swapped variant
    bqa_d = nc.dram_tensor("bqa", [128, DT], F32, kind="ExternalInput").ap()
    bqs_d = nc.dram_tensor("bqs", [128, DT], F32, kind="ExternalInput").ap()
    bka_d = nc.dram_tensor("bka", [128, DT], F32, kind="ExternalInput").ap()
    bks_d = nc.dram_tensor("bks", [128, DT], F32, kind="ExternalInput").ap()
    # row vectors for partition-broadcast loads: bv', ln_g, ln_b
    rows_d = nc.dram_tensor("rows", [3, D], F32, kind="ExternalInput").ap()

    out_d = nc.dram_tensor("out", [N, D], F32, kind="ExternalOutput").ap()
    # bounce rows: [idx] raw sums, [32+idx] reciprocals (reshaped 128-wide)
    rb_d = nc.dram_tensor("rbounce", [H * 4, 512], F32).ap()

    def bcast_row(r):
        return bass.AP(
            tensor=rows_d.tensor, offset=rows_d.offset + r * D, ap=[[0, 128], [1, D]]
        )

    with tile.TileContext(nc) as tc:
        # ---------- pools (two stacks; LIFO release per side) ----------
        # left: whole-kernel + C/D-phase pools; right: A/B- and C-phase pools
        const = tc.alloc_tile_pool(name="const", bufs=1, side="left")
        stp = tc.alloc_tile_pool(name="stp", bufs=4, side="left")
        mvp = tc.alloc_tile_pool(name="mvp", bufs=4, side="left")
        psP = tc.alloc_tile_pool(name="psP", bufs=2, space="PSUM", side="left")
        qkv = tc.alloc_tile_pool(name="qkv", bufs=1, side="right")  # qT,kT,v_sb (A-C)
        trig = tc.alloc_tile_pool(name="trig", bufs=1, side="right")  # cos/sin (A-B)
        ldp = tc.alloc_tile_pool(name="ldp", bufs=2, side="right")
        xnp = tc.alloc_tile_pool(name="xnp", bufs=2, side="right")
        ntp = tc.alloc_tile_pool(name="ntp", bufs=1, side="right")  # snT/xnT shared
        wbp = tc.alloc_tile_pool(name="wbp", bufs=8, side="right")  # wq/wk/wv stream
        rope = tc.alloc_tile_pool(name="rope", bufs=2, side="right")
        bvp = tc.alloc_tile_pool(name="bvp", bufs=1, side="right")
        psT = tc.alloc_tile_pool(name="psT", bufs=2, space="PSUM", side="right")

        # ---- constants
        ident = const.tile([128, 128], F32)
        make_identity(nc, ident)
        eps_t = const.tile([128, 1], F32)
        nc.vector.memset(eps_t, EPS)
        ones128 = const.tile([128, 128], F32)
        nc.vector.memset(ones128, 1.0)
        cosf = trig.tile([128, N], F32)
        nc.sync.dma_start(cosf, cosf_d)
        sinf = trig.tile([128, N], F32)
        nc.sync.dma_start(sinf, sinf_d)
        bqa = const.tile([128, DT], F32)
        nc.sync.dma_start(bqa, bqa_d)
        bqs = const.tile([128, DT], F32)
        nc.sync.dma_start(bqs, bqs_d)
        bka = const.tile([128, DT], F32)
        nc.sync.dma_start(bka, bka_d)
        bks = const.tile([128, DT], F32)
        nc.sync.dma_start(bks, bks_d)
        bv_bc = bvp.tile([128, D], F32)
        nc.sync.dma_start(bv_bc, bcast_row(0))

        # ---- persistent attention operands
        qT = qkv.tile([128, DT, N], F32R, tag="qT")
        kT = qkv.tile([128, DT, N], F32R, tag="kT")
        v_sb = qkv.tile([128, NT, H, 65], F32R, tag="v_sb")
        nc.vector.tensor_copy(
            v_sb[:, :, :, 64:65],
            ones128.rearrange("p (a b c) -> p a b c", a=NT, b=H, c=1),
        )

        def ln_tile(xt, out_ap):
            """LayerNorm [128, D] over free dim -> out_ap."""
            stats = stp.tile([128, 2, 6], F32, tag="stats")
            for g in range(2):
                nc.vector.bn_stats(stats[:, g, :], xt[:, g * 512:(g + 1) * 512])
            mv = mvp.tile([128, 2], F32, tag="mv")
            nc.vector.bn_aggr(mv, stats)
            nc.scalar.activation(
                mv[:, 1:2], mv[:, 1:2], mybir.ActivationFunctionType.Sqrt, bias=eps_t
            )
            nc.vector.reciprocal(mv[:, 1:2], mv[:, 1:2])
            nc.vector.tensor_scalar(
                out=out_ap,
                in0=xt,
                scalar1=mv[:, 0:1],
                scalar2=mv[:, 1:2],
                op0=mybir.AluOpType.subtract,
                op1=mybir.AluOpType.mult,
            )

        def ln_transpose(src_ap, dstT):
            """dstT [128, DT, N] f32r = transpose of LN(src)."""
            for t in range(NT):
                xt = ldp.tile([128, D], F32, tag="ld")
                nc.sync.dma_start(xt, src_ap[t * 128:(t + 1) * 128, :])
                xn = xnp.tile([128, D], F32, tag="xn")
                ln_tile(xt, xn)
                for d in range(DT):
                    pt = psT.tile([128, 128], F32, tag="psT")
                    nc.tensor.transpose(pt, xn[:, d * 128:(d + 1) * 128], ident)
                    nc.scalar.copy(dstT[:, d, t * 128:(t + 1) * 128], pt)

        def load_w(w_d):
            tiles = []
            for dk in range(DT):
                wt = wbp.tile([128, D], F32R, tag="w")
                nc.sync.dma_start(wt, w_d[dk * 128:(dk + 1) * 128, :])
                tiles.append(wt)
            return tiles

        def qk_proj(w_tiles, srcT, dstT, ba, bs):
            """dstT[do, n] = RoPE(W.T @ srcT + bias)."""
            for td in range(DT):
                for hf in range(2):
                    ns = slice(hf * 512, (hf + 1) * 512)
                    ps = psP.tile([128, 512], F32, tag="psP")
                    for dk in range(DT):
                        nc.tensor.matmul(
                            ps,
                            w_tiles[dk][:, td * 128:(td + 1) * 128],
                            srcT[:, dk, ns],
                            start=(dk == 0),
                            stop=(dk == DT - 1),
                        )
                    qs = rope.tile([128, 512], F32, tag="qs")
                    nc.vector.stream_shuffle(qs, ps, _PAIRSWAP)
                    t1 = rope.tile([128, 512], F32, tag="t1")
                    nc.vector.scalar_tensor_tensor(
                        out=t1,
                        in0=ps,
                        scalar=ba[:, td:td + 1],
                        in1=cosf[:, ns],
                        op0=mybir.AluOpType.add,
                        op1=mybir.AluOpType.mult,
                    )
                    t2 = rope.tile([128, 512], F32, tag="t2")
                    nc.vector.scalar_tensor_tensor(
                        out=t2,
                        in0=qs,
                        scalar=bs[:, td:td + 1],
                        in1=sinf[:, ns],
                        op0=mybir.AluOpType.add,
                        op1=mybir.AluOpType.mult,
                    )
                    nc.gpsimd.tensor_add(dstT[:, td, ns], t1, t2)

        # ============ Phase A1+B1: source -> LN -> snT -> qT ============
        snT = ntp.tile([128, DT, N], F32R, tag="nT")
        ln_transpose(s_d, snT)
        wq_t = load_w(wq_d)
        qk_proj(wq_t, snT, qT, bqa, bqs)

        # ============ Phase A2+B2: x -> LN -> xnT -> kT, v ============
        xnT = ntp.tile([128, DT, N], F32R, tag="nT")
        ln_transpose(x_d, xnT)
        wk_t = load_w(wk_d)
        qk_proj(wk_t, xnT, kT, bka, bks)

        wv_t = load_w(wv_d)
        for tt in range(NT):
            for hf in range(2):
                ds_ = slice(hf * 512, (hf + 1) * 512)
                ps = psP.tile([128, 512], F32, tag="psP")
                for dk in range(DT):
                    nc.tensor.matmul(
                        ps,
                        xnT[:, dk, tt * 128:(tt + 1) * 128],
                        wv_t[dk][:, ds_],
                        start=(dk == 0),
                        stop=(dk == DT - 1),
                    )
                nc.vector.tensor_add(
                    v_sb[:, tt, 8 * hf:8 * hf + 8, 0:64],
                    ps.rearrange("p (j d) -> p j d", j=8),
                    bv_bc[:, ds_].rearrange("p (j d) -> p j d", j=8),
                )

        # close A/B-scoped pools (LIFO on right stack)
        for p in (psT, bvp, rope, wbp, ntp, xnp, ldp, trig):
            p.release()

        # ============ Phase C: attention ============
        aop = tc.alloc_tile_pool(name="aop", bufs=1, side="left")  # aoT (C-D)
        wdp = tc.alloc_tile_pool(name="wdp", bufs=8, side="left")  # wo (C-D)
        eTp = tc.alloc_tile_pool(name="eTp", bufs=1, side="right")
        att = tc.alloc_tile_pool(name="att", bufs=2, side="right")
        bcp = tc.alloc_tile_pool(name="bcp", bufs=2, side="right")
        psS = tc.alloc_tile_pool(name="psS", bufs=2, space="PSUM", side="right")
        psO = tc.alloc_tile_pool(name="psO", bufs=2, space="PSUM", side="right")

        aoT = aop.tile([128, DT, N], F32R, tag="aoT")
        wo_t = load_w_pool = None
        wo_t = []
        for dk in range(DT):
            wt = wdp.tile([128, D], F32R, tag="wo")
            nc.sync.dma_start(wt, wo_d[dk * 128:(dk + 1) * 128, :])
            wo_t.append(wt)

        for pair in range(H // 2):
            he, ho = 2 * pair, 2 * pair + 1
            for hf in range(2):
                ns = slice(hf * 512, (hf + 1) * 512)
                # scoresT for both heads of the pair, interleaved so the
                # even/odd matmuls run concurrently on distinct PE row groups
                eps_ = [
                    eTp.tile([128, 2, 2, 512], F32R, tag=f"eT{i}", name=f"ep{i}")
                    for i in range(4)
                ]
                for mb in range(NT):
                    pse = psS.tile([128, 1024], F32, tag="psS")
                    nc.tensor.matmul(
                        pse[:, 0:512],
                        kT[0:64, pair, mb * 128:(mb + 1) * 128],
                        qT[0:64, pair, ns],
                        start=True, stop=True,
                    )
                    nc.tensor.matmul(
                        pse[:, 512:1024],
                        kT[64:128, pair, mb * 128:(mb + 1) * 128],
                        qT[64:128, pair, ns],
                        start=True, stop=True,
                    )
                    nc.scalar.activation(
                        eps_[mb // 2][:, mb % 2], pse,
                        mybir.ActivationFunctionType.Exp, scale=SCALE,
                    )
                # attn @ v for both heads, interleaved accumulation chains
                pso_e = psO.tile([128, 512], F32, tag="psO")
                pso_o = psO.tile([128, 512], F32, tag="psO")
                for mb in range(NT):
                    e_mb = eps_[mb // 2][:, mb % 2]
                    nc.tensor.matmul(
                        pso_e[0:65, :], v_sb[:, mb, he, :], e_mb[:, 0, :],
                        start=(mb == 0), stop=(mb == NT - 1),
                    )
                    nc.tensor.matmul(
                        pso_o[0:65, :], v_sb[:, mb, ho, :], e_mb[:, 1, :],
                        start=(mb == 0), stop=(mb == NT - 1),
                    )
                # normalize: reciprocal denominator, DRAM-bounce broadcast
                for par, pso in ((0, pso_e), (1, pso_o)):
                    idx = 2 * (he + par) + hf
                    r_sb = att.tile([128, 512], F32, tag="r_sb")
                    nc.vector.tensor_copy(r_sb[0:65, :], pso[0:65, :])
                    nc.gpsimd.dma_start(
                        rb_d[idx:idx + 1, :], r_sb[64:65, :]
                    )
                    # reload the 512 sums across 128 partitions, reciprocal
                    # there (8 cyc/elem is per-lane), bounce back, broadcast
                    rt = att.tile([128, 4], F32, tag="rt")
                    nc.gpsimd.dma_start(
                        rt,
                        bass.AP(
                            tensor=rb_d.tensor,
                            offset=rb_d.offset + idx * 512,
                            ap=[[4, 128], [1, 4]],
                        ),
                    )
                    nc.vector.reciprocal(rt, rt)
                    nc.gpsimd.dma_start(
                        bass.AP(
                            tensor=rb_d.tensor,
                            offset=rb_d.offset + (32 + idx) * 512,
                            ap=[[4, 128], [1, 4]],
                        ),
                        rt,
                    )
                    bc = bcp.tile([64, 512], F32, tag="bc")
                    nc.gpsimd.dma_start(
                        bc,
                        bass.AP(
                            tensor=rb_d.tensor,
                            offset=rb_d.offset + (32 + idx) * 512,
                            ap=[[0, 64], [1, 512]],
                        ),
                    )
                    if par == 0:
                        nc.vector.tensor_mul(aoT[0:64, pair, ns], r_sb[0:64, :], bc)
                    else:
                        tmp = att.tile([64, 512], F32R, tag="tmp")
                        nc.vector.tensor_mul(tmp, r_sb[0:64, :], bc)
                        nc.gpsimd.dma_start(aoT[64:128, pair, ns], tmp)

        for p in (psO, psS, bcp, att, eTp, qkv):
            p.release()

        # ============ Phase D: out-proj + final LN ============
        finp = tc.alloc_tile_pool(name="finp", bufs=2, side="left")
        gbp = tc.alloc_tile_pool(name="gbp", bufs=1, side="left")

        g_bc = gbp.tile([128, D], F32, tag="g_bc")
        nc.sync.dma_start(g_bc, bcast_row(1))
        b_bc = gbp.tile([128, D], F32, tag="b_bc")
        nc.sync.dma_start(b_bc, bcast_row(2))

        for nt in range(NT):
            fin = finp.tile([128, D], F32, tag="fin")
            for hf in range(2):
                ds_ = slice(hf * 512, (hf + 1) * 512)
                ps = psP.tile([128, 512], F32, tag="psP")
                for dk in range(DT):
                    nc.tensor.matmul(
                        ps,
                        aoT[:, dk, nt * 128:(nt + 1) * 128],
                        wo_t[dk][:, ds_],
                        start=(dk == 0),
                        stop=(dk == DT - 1),
                    )
                nc.vector.tensor_copy(fin[:, ds_], ps)
            z = finp.tile([128, D], F32, tag="z")
            ln_tile(fin, z)
            nc.vector.tensor_mul(z, z, g_bc)
            nc.vector.tensor_add(z, z, b_bc)
            nc.sync.dma_start(out_d[nt * 128:(nt + 1) * 128, :], z)

        for p in (gbp, finp, wdp, aop, psP, mvp, stp, const):
            p.release()

    nc.compile()
    return nc


_NC_CACHE = None


def _get_nc():
    global _NC_CACHE
    if _NC_CACHE is None:
        _NC_CACHE = build_program()
    return _NC_CACHE


def _host_prep(inputs):
    f64 = np.float64
    Wq = inputs["Wq"].astype(f64)
    Wk = inputs["Wk"].astype(f64)
    Wv = inputs["Wv"].astype(f64)

    wq = (inputs["nq_g"].astype(f64)[:, None] * Wq).astype(np.float32)
    wk = (inputs["nk_g"].astype(f64)[:, None] * Wk).astype(np.float32)
    wv = (inputs["nv_g"].astype(f64)[:, None] * Wv).astype(np.float32)
    bq = (inputs["nq_b"].astype(f64) @ Wq + inputs["bq"].astype(f64)).astype(np.float32)
    bk = (inputs["nk_b"].astype(f64) @ Wk + inputs["bk"].astype(f64)).astype(np.float32)
    bv = (inputs["nv_b"].astype(f64) @ Wv + inputs["bv"].astype(f64)).astype(np.float32)

    # rope tables
    freqs = (1.0 / THETA ** (np.arange(0, DH, 2, dtype=np.float32) / DH)).astype(
        np.float32
    )
    t = np.arange(N, dtype=np.float32)
    ang = np.outer(t, freqs).astype(np.float64)  # [N, 32]
    cos_t = np.cos(ang).astype(np.float32)
    sin_t = np.sin(ang).astype(np.float32)
    p = np.arange(128)
    i_of_p = (p % 64) // 2
    cosf = np.ascontiguousarray(cos_t[:, i_of_p].T)  # [128, N]
    sgn = np.where(p % 2 == 0, -1.0, 1.0).astype(np.float32)
    sinf = np.ascontiguousarray(sin_t[:, i_of_p].T * sgn[:, None]).astype(np.float32)

    def btab(b):
        tab = np.zeros((128, DT), np.float32)
        tabs = np.zeros((128, DT), np.float32)
        for td in range(DT):
            tab[:, td] = b[td * 128 + p]
            tabs[:, td] = b[td * 128 + (p ^ 1)]
        return tab, tabs

    bqa, bqs = btab(bq)
    bka, bks = btab(bk)

    rows = np.stack(
        [bv, inputs["ln_g"].astype(np.float32), inputs["ln_b"].astype(np.float32)]
    )

    return {
        "wq": wq, "wk": wk, "wv": wv,
        "wo": np.ascontiguousarray(inputs["Wo"].astype(np.float32)),
        "cosf": cosf, "sinf": sinf,
        "bqa": bqa, "bqs": bqs, "bka": bka, "bks": bks,
        "rows": rows.astype(np.float32),
    }


def run(inputs, trace=False, tmpdir=None):
    nc = _get_nc()
    shared = _host_prep(inputs)
    x = np.asarray(inputs["x"], np.float32)
    src = np.asarray(inputs["source"], np.float32)
    in_maps = [
        {"x": np.ascontiguousarray(x[c]), "src": np.ascontiguousarray(src[c]), **shared}
        for c in range(B)
    ]
    res = run_bass_kernel_spmd(nc, in_maps, list(range(B)), trace=trace, tmpdir=tmpdir)
    out = np.stack([res.results[c]["out"] for c in range(B)]).astype(np.float32)
    return out, res


def kernel(**inputs):
    return run(inputs)[0]


# revision 23
# speedup vs baseline: 1.0430x; 1.0430x over previous
"""Trainium2 Bass kernel for nn_AttentionLayer (B=8, N=1024, D=1024, H=16).

Sharding: data-parallel over batch -- one batch element per NeuronCore (8 cores).

Per-core plan (activations kept transposed [D, tokens] so every matmul maps
natively onto the PE array's partition-dim contraction):
  1. LayerNorm x / source in natural layout (bn_stats over free dim); gains and
     biases of the pre-norms are folded into the projection weights on the host.
  2. PE-transpose the normalized activations -> snT/xnT [D, tok].
  3. Projections: qT/kT = W.T @ snT|xnT (transposed out; bias + RoPE fused into
     the PSUM epilogue via stream_shuffle + scalar_tensor_tensor);
     v natural [tok, D] via xnT-stationary matmul, ones column packed per head.
  4. Attention per head: scoresT = kT-block stationary x qT moving; exp on ACT
     with fused 1/sqrt(dh) scale; attn@v with v|ones stationary yields both
     output^T and the softmax denominator in one PSUM accumulation chain.
  5. Normalize by the reciprocal denominator (DRAM-bounce partition broadcast),
     out-projection with attn_outT stationary producing the NATURAL-layout
     result, final LayerNorm in natural layout, store.
All matmul operands are float32r (TF32-class, full PE rate at N=512).
"""

import numpy as np

import concourse.bass as bass
import concourse.tile as tile
from concourse import bacc, mybir
from concourse.bass_utils import run_bass_kernel_spmd
from concourse.masks import make_identity

F32 = mybir.dt.float32
F32R = mybir.dt.float32r
BF16 = mybir.dt.bfloat16
import os as _os
MM_DT = BF16 if _os.environ.get("KERNEL_MM_BF16", "0") == "1" else F32R

B, N, D, H = 8, 1024, 1024, 16
DH = D // H  # 64
EPS = 1e-5
THETA = 10000.0
NT = N // 128  # 8 token tiles
DT = D // 128  # 8 channel tiles
SCALE = float(DH) ** -0.5

_PAIRSWAP = []
for _i in range(16):
    _PAIRSWAP += [2 * _i + 1, 2 * _i]


def build_program():
    nc = bacc.Bacc("TRN2", target_bir_lowering=False, debug=False)

    x_d = nc.dram_tensor("x", [N, D], F32, kind="ExternalInput").ap()
    s_d = nc.dram_tensor("src", [N, D], F32, kind="ExternalInput").ap()
    wq_d = nc.dram_tensor("wq", [D, D], MM_DT, kind="ExternalInput").ap()
    wk_d = nc.dram_tensor("wk", [D, D], MM_DT, kind="ExternalInput").ap()
    wv_d = nc.dram_tensor("wv", [D, D], MM_DT, kind="ExternalInput").ap()
    wo_d = nc.dram_tensor("wo", [D, D], MM_DT, kind="ExternalInput").ap()
    cosf_d = nc.dram_tensor("cosf", [128, N], F32, kind="ExternalInput").ap()
    sinf_d = nc.dram_tensor("sinf", [128, N], F32, kind="ExternalInput").ap()
    # bias tables [128, DT] (col t = do-tile t); *s = pair-swapped variant
    bqa_d = nc.dram_tensor("bqa", [128, DT], F32, kind="ExternalInput").ap()
    bqs_d = nc.dram_tensor("bqs", [128, DT], F32, kind="ExternalInput").ap()
    bka_d = nc.dram_tensor("bka", [128, DT], F32, kind="ExternalInput").ap()
    bks_d = nc.dram_tensor("bks", [128, DT], F32, kind="ExternalInput").ap()
    # row vectors for partition-broadcast loads: bv', ln_g, ln_b
    rows_d = nc.dram_tensor("rows", [3, D], F32, kind="ExternalInput").ap()

    out_d = nc.dram_tensor("out", [N, D], F32, kind="ExternalOutput").ap()
    # bounce rows: [idx] raw sums, [32+idx] reciprocals (reshaped 128-wide)
    rb_d = nc.dram_tensor("rbounce", [H * 4, 512], F32).ap()

    def bcast_row(r):
        return bass.AP(
            tensor=rows_d.tensor, offset=rows_d.offset + r * D, ap=[[0, 128], [1, D]]
        )

    with tile.TileContext(nc) as tc:
        # ---------- pools (two stacks; LIFO release per side) ----------
        # left: whole-kernel + C/D-phase pools; right: A/B- and C-phase pools
        const = tc.alloc_tile_pool(name="const", bufs=1, side="left")
        stp = tc.alloc_tile_pool(name="stp", bufs=4, side="left")
        mvp = tc.alloc_tile_pool(name="mvp", bufs=4, side="left")
        psP = tc.alloc_tile_pool(name="psP", bufs=3, space="PSUM", side="left")
        qkv = tc.alloc_tile_pool(name="qkv", bufs=1, side="right")  # qT,kT,v_sb (A-C)
        trig = tc.alloc_tile_pool(name="trig", bufs=1, side="right")  # cos/sin (A-B)
        ldp = tc.alloc_tile_pool(name="ldp", bufs=2, side="right")
        xnp = tc.alloc_tile_pool(name="xnp", bufs=2, side="right")
        ntp = tc.alloc_tile_pool(name="ntp", bufs=1, side="right")  # snT/xnT shared
        wbp = tc.alloc_tile_pool(name="wbp", bufs=8, side="right")  # wq/wk/wv stream
        rope = tc.alloc_tile_pool(name="rope", bufs=2, side="right")
        bvp = tc.alloc_tile_pool(name="bvp", bufs=1, side="right")
        psT = tc.alloc_tile_pool(name="psT", bufs=2, space="PSUM", side="right")

        # ---- constants
        ident = const.tile([128, 128], F32)
        make_identity(nc, ident)
        eps_t = const.tile([128, 1], F32)
        nc.vector.memset(eps_t, EPS)
        ones128 = const.tile([128, 128], F32)
        nc.vector.memset(ones128, 1.0)
        cosf = trig.tile([128, N], F32)
        nc.sync.dma_start(cosf, cosf_d)
        sinf = trig.tile([128, N], F32)
        nc.sync.dma_start(sinf, sinf_d)
        bqa = const.tile([128, DT], F32)
        nc.sync.dma_start(bqa, bqa_d)
        bqs = const.tile([128, DT], F32)
        nc.sync.dma_start(bqs, bqs_d)
        bka = const.tile([128, DT], F32)
        nc.sync.dma_start(bka, bka_d)
        bks = const.tile([128, DT], F32)
        nc.sync.dma_start(bks, bks_d)
        bv_bc = bvp.tile([128, D], F32)
        nc.sync.dma_start(bv_bc, bcast_row(0))

        # ---- persistent attention operands
        qT = qkv.tile([128, DT, N], MM_DT, tag="qT")
        kT = qkv.tile([128, DT, N], MM_DT, tag="kT")
        v_sb = qkv.tile([128, NT, H, 65], MM_DT, tag="v_sb")
        nc.vector.tensor_copy(
            v_sb[:, :, :, 64:65],
            ones128.rearrange("p (a b c) -> p a b c", a=NT, b=H, c=1),
        )

        def ln_tile(xt, out_ap):
            """LayerNorm [128, D] over free dim -> out_ap."""
            stats = stp.tile([128, 2, 6], F32, tag="stats")
            for g in range(2):
                nc.vector.bn_stats(stats[:, g, :], xt[:, g * 512:(g + 1) * 512])
            mv = mvp.tile([128, 2], F32, tag="mv")
            nc.vector.bn_aggr(mv, stats)
            nc.scalar.activation(
                mv[:, 1:2], mv[:, 1:2], mybir.ActivationFunctionType.Sqrt, bias=eps_t
            )
            nc.vector.reciprocal(mv[:, 1:2], mv[:, 1:2])
            nc.vector.tensor_scalar(
                out=out_ap,
                in0=xt,
                scalar1=mv[:, 0:1],
                scalar2=mv[:, 1:2],
                op0=mybir.AluOpType.subtract,
                op1=mybir.AluOpType.mult,
            )

        def ln_transpose(src_ap, dstT):
            """dstT [128, DT, N] f32r = transpose of LN(src)."""
            for t in range(NT):
                xt = ldp.tile([128, D], F32, tag="ld")
                nc.sync.dma_start(xt, src_ap[t * 128:(t + 1) * 128, :])
                xn = xnp.tile([128, D], F32, tag="xn")
                ln_tile(xt, xn)
                for d in range(DT):
                    pt = psT.tile([128, 128], F32, tag="psT")
                    nc.tensor.transpose(pt, xn[:, d * 128:(d + 1) * 128], ident)
                    nc.scalar.copy(dstT[:, d, t * 128:(t + 1) * 128], pt)

        def load_w(w_d):
            tiles = []
            for dk in range(DT):
                wt = wbp.tile([128, D], MM_DT, tag="w")
                nc.sync.dma_start(wt, w_d[dk * 128:(dk + 1) * 128, :])
                tiles.append(wt)
            return tiles

        def qk_proj(w_tiles, srcT, dstT, ba, bs):
            """dstT[do, n] = RoPE(W.T @ srcT + bias)."""
            for td in range(DT):
                for hf in range(2):
                    ns = slice(hf * 512, (hf + 1) * 512)
                    ps = psP.tile([128, 512], F32, tag="psP")
                    for dk in range(DT):
                        nc.tensor.matmul(
                            ps,
                            w_tiles[dk][:, td * 128:(td + 1) * 128],
                            srcT[:, dk, ns],
                            start=(dk == 0),
                            stop=(dk == DT - 1),
                        )
                    qs = rope.tile([128, 512], F32, tag="qs")
                    nc.vector.stream_shuffle(qs, ps, _PAIRSWAP)
                    t1 = rope.tile([128, 512], F32, tag="t1")
                    nc.vector.scalar_tensor_tensor(
                        out=t1,
                        in0=ps,
                        scalar=ba[:, td:td + 1],
                        in1=cosf[:, ns],
                        op0=mybir.AluOpType.add,
                        op1=mybir.AluOpType.mult,
                    )
                    t2 = rope.tile([128, 512], F32, tag="t2")
                    nc.vector.scalar_tensor_tensor(
                        out=t2,
                        in0=qs,
                        scalar=bs[:, td:td + 1],
                        in1=sinf[:, ns],
                        op0=mybir.AluOpType.add,
                        op1=mybir.AluOpType.mult,
                    )
                    nc.gpsimd.tensor_add(dstT[:, td, ns], t1, t2)

        # ============ Phase A1+B1: source -> LN -> snT -> qT ============
        snT = ntp.tile([128, DT, N], MM_DT, tag="nT")
        ln_transpose(s_d, snT)
        wq_t = load_w(wq_d)
        qk_proj(wq_t, snT, qT, bqa, bqs)

        # ============ Phase A2+B2: x -> LN -> xnT -> kT, v ============
        xnT = ntp.tile([128, DT, N], MM_DT, tag="nT")
        ln_transpose(x_d, xnT)
        wk_t = load_w(wk_d)
        qk_proj(wk_t, xnT, kT, bka, bks)

        wv_t = load_w(wv_d)
        for tt in range(NT):
            for hf in range(2):
                ds_ = slice(hf * 512, (hf + 1) * 512)
                ps = psP.tile([128, 512], F32, tag="psP")
                for dk in range(DT):
                    nc.tensor.matmul(
                        ps,
                        xnT[:, dk, tt * 128:(tt + 1) * 128],
                        wv_t[dk][:, ds_],
                        start=(dk == 0),
                        stop=(dk == DT - 1),
                    )
                nc.vector.tensor_add(
                    v_sb[:, tt, 8 * hf:8 * hf + 8, 0:64],
                    ps.rearrange("p (j d) -> p j d", j=8),
                    bv_bc[:, ds_].rearrange("p (j d) -> p j d", j=8),
                )

        # close A/B-scoped pools (LIFO on right stack)
        for p in (psT, bvp, rope, wbp, ntp, xnp, ldp, trig):
            p.release()

        # ============ Phase C: attention ============
        psP.release()
        aop = tc.alloc_tile_pool(name="aop", bufs=1, side="left")  # aoT (C-D)
        wdp = tc.alloc_tile_pool(name="wdp", bufs=8, side="left")  # wo (C-D)
        eTp = tc.alloc_tile_pool(name="eTp", bufs=1, side="right")
        att = tc.alloc_tile_pool(name="att", bufs=2, side="right")
        bcp = tc.alloc_tile_pool(name="bcp", bufs=2, side="right")
        psS = tc.alloc_tile_pool(name="psS", bufs=2, space="PSUM", side="right")
        psO = tc.alloc_tile_pool(name="psO", bufs=3, space="PSUM", side="right")

        aoT = aop.tile([128, DT, N], MM_DT, tag="aoT")
        wo_t = load_w_pool = None
        wo_t = []
        for dk in range(DT):
            wt = wdp.tile([128, D], MM_DT, tag="wo")
            nc.sync.dma_start(wt, wo_d[dk * 128:(dk + 1) * 128, :])
            wo_t.append(wt)

        for pair in range(H // 2):
            he, ho = 2 * pair, 2 * pair + 1
            for hf in range(2):
                ns = slice(hf * 512, (hf + 1) * 512)
                # scoresT for both heads of the pair, interleaved so the
                # even/odd matmuls run concurrently on distinct PE row groups
                eps_ = [
                    eTp.tile([128, 2, 2, 512], MM_DT, tag=f"eT{i}", name=f"ep{i}")
                    for i in range(4)
                ]
                for mb in range(NT):
                    pse = psS.tile([128, 1024], F32, tag="psS")
                    nc.tensor.matmul(
                        pse[:, 0:512],
                        kT[0:64, pair, mb * 128:(mb + 1) * 128],
                        qT[0:64, pair, ns],
                        start=True, stop=True,
                    )
                    nc.tensor.matmul(
                        pse[:, 512:1024],
                        kT[64:128, pair, mb * 128:(mb + 1) * 128],
                        qT[64:128, pair, ns],
                        start=True, stop=True,
                    )
                    nc.scalar.activation(
                        eps_[mb // 2][:, mb % 2], pse,
                        mybir.ActivationFunctionType.Exp, scale=SCALE,
                    )
                # attn @ v for both heads, interleaved accumulation chains
                pso_e = psO.tile([128, 512], F32, tag="psO")
                pso_o = psO.tile([128, 512], F32, tag="psO")
                for mb in range(NT):
                    e_mb = eps_[mb // 2][:, mb % 2]
                    nc.tensor.matmul(
                        pso_e[0:65, :], v_sb[:, mb, he, :], e_mb[:, 0, :],
                        start=(mb == 0), stop=(mb == NT - 1),
                    )
                    nc.tensor.matmul(
                        pso_o[0:65, :], v_sb[:, mb, ho, :], e_mb[:, 1, :],
                        start=(mb == 0), stop=(mb == NT - 1),
                    )
                # normalize: reciprocal denominator, DRAM-bounce broadcast
                for par, pso in ((0, pso_e), (1, pso_o)):
                    idx = 2 * (he + par) + hf
                    r_sb = att.tile([128, 512], F32, tag="r_sb")
                    nc.vector.tensor_copy(r_sb[0:65, :], pso[0:65, :])
                    nc.gpsimd.dma_start(
                        rb_d[idx:idx + 1, :], r_sb[64:65, :]
                    )
                    # reload the 512 sums across 128 partitions, reciprocal
                    # there (8 cyc/elem is per-lane), bounce back, broadcast
                    rt = att.tile([128, 4], F32, tag="rt")
                    nc.gpsimd.dma_start(
                        rt,
                        bass.AP(
                            tensor=rb_d.tensor,
                            offset=rb_d.offset + idx * 512,
                            ap=[[4, 128], [1, 4]],
                        ),
                    )
                    nc.vector.reciprocal(rt, rt)
                    nc.gpsimd.dma_start(
                        bass.AP(
                            tensor=rb_d.tensor,
                            offset=rb_d.offset + (32 + idx) * 512,
                            ap=[[4, 128], [1, 4]],
                        ),
                        rt,
                    )
                    bc = bcp.tile([64, 512], F32, tag="bc")
                    nc.gpsimd.dma_start(
                        bc,
                        bass.AP(
                            tensor=rb_d.tensor,
                            offset=rb_d.offset + (32 + idx) * 512,
                            ap=[[0, 64], [1, 512]],
                        ),
                    )
                    if par == 0:
                        nc.vector.tensor_mul(aoT[0:64, pair, ns], r_sb[0:64, :], bc)
                    else:
                        tmp = att.tile([64, 512], MM_DT, tag="tmp")
                        nc.vector.tensor_mul(tmp, r_sb[0:64, :], bc)
                        nc.gpsimd.dma_start(aoT[64:128, pair, ns], tmp)

        for p in (psO, psS, bcp, att, eTp, qkv):
            p.release()

        # ============ Phase D: out-proj + final LN ============
        finp = tc.alloc_tile_pool(name="finp", bufs=2, side="left")
        gbp = tc.alloc_tile_pool(name="gbp", bufs=1, side="left")
        psD = tc.alloc_tile_pool(name="psD", bufs=2, space="PSUM", side="left")

        g_bc = gbp.tile([128, D], F32, tag="g_bc")
        nc.sync.dma_start(g_bc, bcast_row(1))
        b_bc = gbp.tile([128, D], F32, tag="b_bc")
        nc.sync.dma_start(b_bc, bcast_row(2))

        for nt in range(NT):
            fin = finp.tile([128, D], F32, tag="fin")
            for hf in range(2):
                ds_ = slice(hf * 512, (hf + 1) * 512)
                ps = psD.tile([128, 512], F32, tag="psD")
                for dk in range(DT):
                    nc.tensor.matmul(
                        ps,
                        aoT[:, dk, nt * 128:(nt + 1) * 128],
                        wo_t[dk][:, ds_],
                        start=(dk == 0),
                        stop=(dk == DT - 1),
                    )
                nc.vector.tensor_copy(fin[:, ds_], ps)
            z = finp.tile([128, D], F32, tag="z")
            ln_tile(fin, z)
            nc.vector.tensor_mul(z, z, g_bc)
            nc.vector.tensor_add(z, z, b_bc)
            nc.sync.dma_start(out_d[nt * 128:(nt + 1) * 128, :], z)

        for p in (psD, gbp, finp, wdp, aop, mvp, stp, const):
            p.release()

    nc.compile()
    return nc


_NC_CACHE = None


def _get_nc():
    global _NC_CACHE
    if _NC_CACHE is None:
        _NC_CACHE = build_program()
    return _NC_CACHE


def _host_prep(inputs):
    f64 = np.float64
    Wq = inputs["Wq"].astype(f64)
    Wk = inputs["Wk"].astype(f64)
    Wv = inputs["Wv"].astype(f64)

    wq = (inputs["nq_g"].astype(f64)[:, None] * Wq).astype(np.float32)
    wk = (inputs["nk_g"].astype(f64)[:, None] * Wk).astype(np.float32)
    wv = (inputs["nv_g"].astype(f64)[:, None] * Wv).astype(np.float32)
    bq = (inputs["nq_b"].astype(f64) @ Wq + inputs["bq"].astype(f64)).astype(np.float32)
    bk = (inputs["nk_b"].astype(f64) @ Wk + inputs["bk"].astype(f64)).astype(np.float32)
    bv = (inputs["nv_b"].astype(f64) @ Wv + inputs["bv"].astype(f64)).astype(np.float32)

    # rope tables
    freqs = (1.0 / THETA ** (np.arange(0, DH, 2, dtype=np.float32) / DH)).astype(
        np.float32
    )
    t = np.arange(N, dtype=np.float32)
    ang = np.outer(t, freqs).astype(np.float64)  # [N, 32]
    cos_t = np.cos(ang).astype(np.float32)
    sin_t = np.sin(ang).astype(np.float32)
    p = np.arange(128)
    i_of_p = (p % 64) // 2
    cosf = np.ascontiguousarray(cos_t[:, i_of_p].T)  # [128, N]
    sgn = np.where(p % 2 == 0, -1.0, 1.0).astype(np.float32)
    sinf = np.ascontiguousarray(sin_t[:, i_of_p].T * sgn[:, None]).astype(np.float32)

    def btab(b):
        tab = np.zeros((128, DT), np.float32)
        tabs = np.zeros((128, DT), np.float32)
        for td in range(DT):
            tab[:, td] = b[td * 128 + p]
            tabs[:, td] = b[td * 128 + (p ^ 1)]
        return tab, tabs

    bqa, bqs = btab(bq)
    bka, bks = btab(bk)

    rows = np.stack(
        [bv, inputs["ln_g"].astype(np.float32), inputs["ln_b"].astype(np.float32)]
    )

    if MM_DT == BF16:
        import ml_dtypes
        wire = ml_dtypes.bfloat16
        wq, wk, wv = (a.astype(wire) for a in (wq, wk, wv))
        wo_w = np.ascontiguousarray(inputs["Wo"].astype(np.float64)).astype(wire)
    else:
        wo_w = np.ascontiguousarray(inputs["Wo"].astype(np.float32))
    return {
        "wq": wq, "wk": wk, "wv": wv,
        "wo": wo_w,
        "cosf": cosf, "sinf": sinf,
        "bqa": bqa, "bqs": bqs, "bka": bka, "bks": bks,
        "rows": rows.astype(np.float32),
    }


def run(inputs, trace=False, tmpdir=None):
    nc = _get_nc()
    shared = _host_prep(inputs)
    x = np.asarray(inputs["x"], np.float32)
    src = np.asarray(inputs["source"], np.float32)
    in_maps = [
        {"x": np.ascontiguousarray(x[c]), "src": np.ascontiguousarray(src[c]), **shared}
        for c in range(B)
    ]
    res = run_bass_kernel_spmd(nc, in_maps, list(range(B)), trace=trace, tmpdir=tmpdir)
    out = np.stack([res.results[c]["out"] for c in range(B)]).astype(np.float32)
    return out, res


def kernel(**inputs):
    return run(inputs)[0]


# revision 24
# speedup vs baseline: 1.2380x; 1.1869x over previous
"""Trainium2 Bass kernel for nn_AttentionLayer (B=8, N=1024, D=1024, H=16).

Sharding: data-parallel over batch -- one batch element per NeuronCore (8 cores).

Per-core plan (activations kept transposed [D, tokens] so every matmul maps
natively onto the PE array's partition-dim contraction):
  1. LayerNorm x / source in natural layout (bn_stats over free dim); gains and
     biases of the pre-norms are folded into the projection weights on the host.
  2. PE-transpose the normalized activations -> snT/xnT [D, tok].
  3. Projections: qT/kT = W.T @ snT|xnT (transposed out; bias + RoPE fused into
     the PSUM epilogue via stream_shuffle + scalar_tensor_tensor);
     v natural [tok, D] via xnT-stationary matmul, ones column packed per head.
  4. Attention per head: scoresT = kT-block stationary x qT moving; exp on ACT
     with fused 1/sqrt(dh) scale; attn@v with v|ones stationary yields both
     output^T and the softmax denominator in one PSUM accumulation chain.
  5. Normalize by the reciprocal denominator (DRAM-bounce partition broadcast),
     out-projection with attn_outT stationary producing the NATURAL-layout
     result, final LayerNorm in natural layout, store.
All matmul operands are float32r (TF32-class, full PE rate at N=512).
"""

import numpy as np

import concourse.bass as bass
import concourse.tile as tile
from concourse import bacc, mybir
from concourse.bass_utils import run_bass_kernel_spmd
from concourse.masks import make_identity

F32 = mybir.dt.float32
F32R = mybir.dt.float32r
BF16 = mybir.dt.bfloat16
import os as _os
MM_DT = BF16 if _os.environ.get("KERNEL_MM_BF16", "0") == "1" else F32R
DEEP = MM_DT == BF16  # bf16 halves the big tiles; spend it on pipeline depth

B, N, D, H = 8, 1024, 1024, 16
DH = D // H  # 64
EPS = 1e-5
THETA = 10000.0
NT = N // 128  # 8 token tiles
DT = D // 128  # 8 channel tiles
SCALE = float(DH) ** -0.5

_PAIRSWAP = []
for _i in range(16):
    _PAIRSWAP += [2 * _i + 1, 2 * _i]


def build_program():
    nc = bacc.Bacc("TRN2", target_bir_lowering=False, debug=False)

    x_d = nc.dram_tensor("x", [N, D], F32, kind="ExternalInput").ap()
    s_d = nc.dram_tensor("src", [N, D], F32, kind="ExternalInput").ap()
    wq_d = nc.dram_tensor("wq", [D, D], MM_DT, kind="ExternalInput").ap()
    wk_d = nc.dram_tensor("wk", [D, D], MM_DT, kind="ExternalInput").ap()
    wv_d = nc.dram_tensor("wv", [D, D], MM_DT, kind="ExternalInput").ap()
    wo_d = nc.dram_tensor("wo", [D, D], MM_DT, kind="ExternalInput").ap()
    cosf_d = nc.dram_tensor("cosf", [128, N], F32, kind="ExternalInput").ap()
    sinf_d = nc.dram_tensor("sinf", [128, N], F32, kind="ExternalInput").ap()
    # bias tables [128, DT] (col t = do-tile t); *s = pair-swapped variant
    bqa_d = nc.dram_tensor("bqa", [128, DT], F32, kind="ExternalInput").ap()
    bqs_d = nc.dram_tensor("bqs", [128, DT], F32, kind="ExternalInput").ap()
    bka_d = nc.dram_tensor("bka", [128, DT], F32, kind="ExternalInput").ap()
    bks_d = nc.dram_tensor("bks", [128, DT], F32, kind="ExternalInput").ap()
    # row vectors for partition-broadcast loads: bv', ln_g, ln_b
    rows_d = nc.dram_tensor("rows", [3, D], F32, kind="ExternalInput").ap()

    out_d = nc.dram_tensor("out", [N, D], F32, kind="ExternalOutput").ap()
    # bounce rows: [idx] raw sums, [32+idx] reciprocals (reshaped 128-wide)
    rb_d = nc.dram_tensor("rbounce", [H * 4, 512], F32).ap()

    def bcast_row(r):
        return bass.AP(
            tensor=rows_d.tensor, offset=rows_d.offset + r * D, ap=[[0, 128], [1, D]]
        )

    with tile.TileContext(nc) as tc:
        # ---------- pools (two stacks; LIFO release per side) ----------
        # left: whole-kernel + C/D-phase pools; right: A/B- and C-phase pools
        const = tc.alloc_tile_pool(name="const", bufs=1, side="left")
        stp = tc.alloc_tile_pool(name="stp", bufs=4, side="left")
        mvp = tc.alloc_tile_pool(name="mvp", bufs=4, side="left")
        psP = tc.alloc_tile_pool(name="psP", bufs=4, space="PSUM", side="left")
        qkv = tc.alloc_tile_pool(name="qkv", bufs=1, side="right")  # qT,kT,v_sb (A-C)
        trig = tc.alloc_tile_pool(name="trig", bufs=1, side="right")  # cos/sin (A-B)
        ldp = tc.alloc_tile_pool(name="ldp", bufs=3, side="right")
        xnp = tc.alloc_tile_pool(name="xnp", bufs=2, side="right")
        ntp = tc.alloc_tile_pool(name="ntp", bufs=2 if DEEP else 1, side="right")
        wbp = tc.alloc_tile_pool(name="wbp", bufs=8, side="right")  # wq/wk/wv stream
        rope = tc.alloc_tile_pool(name="rope", bufs=3, side="right")
        bvp = tc.alloc_tile_pool(name="bvp", bufs=1, side="right")
        psT = tc.alloc_tile_pool(name="psT", bufs=2, space="PSUM", side="right")

        # ---- constants
        ident = const.tile([128, 128], F32)
        make_identity(nc, ident)
        eps_t = const.tile([128, 1], F32)
        nc.vector.memset(eps_t, EPS)
        ones128 = const.tile([128, 128], F32)
        nc.vector.memset(ones128, 1.0)
        cosf = trig.tile([128, N], F32)
        nc.sync.dma_start(cosf, cosf_d)
        sinf = trig.tile([128, N], F32)
        nc.sync.dma_start(sinf, sinf_d)
        bqa = const.tile([128, DT], F32)
        nc.sync.dma_start(bqa, bqa_d)
        bqs = const.tile([128, DT], F32)
        nc.sync.dma_start(bqs, bqs_d)
        bka = const.tile([128, DT], F32)
        nc.sync.dma_start(bka, bka_d)
        bks = const.tile([128, DT], F32)
        nc.sync.dma_start(bks, bks_d)
        bv_bc = bvp.tile([128, D], F32)
        nc.sync.dma_start(bv_bc, bcast_row(0))

        # ---- persistent attention operands
        qT = qkv.tile([128, DT, N], MM_DT, tag="qT")
        kT = qkv.tile([128, DT, N], MM_DT, tag="kT")
        v_sb = qkv.tile([128, NT, H, 65], MM_DT, tag="v_sb")
        nc.vector.tensor_copy(
            v_sb[:, :, :, 64:65],
            ones128.rearrange("p (a b c) -> p a b c", a=NT, b=H, c=1),
        )

        def ln_tile(xt, out_ap):
            """LayerNorm [128, D] over free dim -> out_ap."""
            stats = stp.tile([128, 2, 6], F32, tag="stats")
            for g in range(2):
                nc.vector.bn_stats(stats[:, g, :], xt[:, g * 512:(g + 1) * 512])
            mv = mvp.tile([128, 2], F32, tag="mv")
            nc.vector.bn_aggr(mv, stats)
            nc.scalar.activation(
                mv[:, 1:2], mv[:, 1:2], mybir.ActivationFunctionType.Sqrt, bias=eps_t
            )
            nc.vector.reciprocal(mv[:, 1:2], mv[:, 1:2])
            nc.vector.tensor_scalar(
                out=out_ap,
                in0=xt,
                scalar1=mv[:, 0:1],
                scalar2=mv[:, 1:2],
                op0=mybir.AluOpType.subtract,
                op1=mybir.AluOpType.mult,
            )

        def ln_transpose(src_ap, dstT):
            """dstT [128, DT, N] f32r = transpose of LN(src)."""
            for t in range(NT):
                xt = ldp.tile([128, D], F32, tag="ld")
                nc.sync.dma_start(xt, src_ap[t * 128:(t + 1) * 128, :])
                xn = xnp.tile([128, D], F32, tag="xn")
                ln_tile(xt, xn)
                for d in range(DT):
                    pt = psT.tile([128, 128], F32, tag="psT")
                    nc.tensor.transpose(pt, xn[:, d * 128:(d + 1) * 128], ident)
                    nc.scalar.copy(dstT[:, d, t * 128:(t + 1) * 128], pt)

        def load_w(w_d):
            tiles = []
            for dk in range(DT):
                wt = wbp.tile([128, D], MM_DT, tag="w")
                nc.sync.dma_start(wt, w_d[dk * 128:(dk + 1) * 128, :])
                tiles.append(wt)
            return tiles

        def qk_proj(w_tiles, srcT, dstT, ba, bs):
            """dstT[do, n] = RoPE(W.T @ srcT + bias)."""
            for td in range(DT):
                for hf in range(2):
                    ns = slice(hf * 512, (hf + 1) * 512)
                    ps = psP.tile([128, 512], F32, tag="psP")
                    for dk in range(DT):
                        nc.tensor.matmul(
                            ps,
                            w_tiles[dk][:, td * 128:(td + 1) * 128],
                            srcT[:, dk, ns],
                            start=(dk == 0),
                            stop=(dk == DT - 1),
                        )
                    qs = rope.tile([128, 512], F32, tag="qs")
                    nc.vector.stream_shuffle(qs, ps, _PAIRSWAP)
                    t1 = rope.tile([128, 512], F32, tag="t1")
                    nc.vector.scalar_tensor_tensor(
                        out=t1,
                        in0=ps,
                        scalar=ba[:, td:td + 1],
                        in1=cosf[:, ns],
                        op0=mybir.AluOpType.add,
                        op1=mybir.AluOpType.mult,
                    )
                    t2 = rope.tile([128, 512], F32, tag="t2")
                    nc.vector.scalar_tensor_tensor(
                        out=t2,
                        in0=qs,
                        scalar=bs[:, td:td + 1],
                        in1=sinf[:, ns],
                        op0=mybir.AluOpType.add,
                        op1=mybir.AluOpType.mult,
                    )
                    nc.gpsimd.tensor_add(dstT[:, td, ns], t1, t2)

        # ============ Phase A1+B1: source -> LN -> snT -> qT ============
        snT = ntp.tile([128, DT, N], MM_DT, tag="nT")
        ln_transpose(s_d, snT)
        wq_t = load_w(wq_d)
        qk_proj(wq_t, snT, qT, bqa, bqs)

        # ============ Phase A2+B2: x -> LN -> xnT -> kT, v ============
        xnT = ntp.tile([128, DT, N], MM_DT, tag="nT")
        ln_transpose(x_d, xnT)
        wk_t = load_w(wk_d)
        qk_proj(wk_t, xnT, kT, bka, bks)

        wv_t = load_w(wv_d)
        for tt in range(NT):
            for hf in range(2):
                ds_ = slice(hf * 512, (hf + 1) * 512)
                ps = psP.tile([128, 512], F32, tag="psP")
                for dk in range(DT):
                    nc.tensor.matmul(
                        ps,
                        xnT[:, dk, tt * 128:(tt + 1) * 128],
                        wv_t[dk][:, ds_],
                        start=(dk == 0),
                        stop=(dk == DT - 1),
                    )
                nc.vector.tensor_add(
                    v_sb[:, tt, 8 * hf:8 * hf + 8, 0:64],
                    ps.rearrange("p (j d) -> p j d", j=8),
                    bv_bc[:, ds_].rearrange("p (j d) -> p j d", j=8),
                )

        # close A/B-scoped pools (LIFO on right stack)
        for p in (psT, bvp, rope, wbp, ntp, xnp, ldp, trig):
            p.release()

        # ============ Phase C: attention ============
        psP.release()
        aop = tc.alloc_tile_pool(name="aop", bufs=1, side="left")  # aoT (C-D)
        wdp = tc.alloc_tile_pool(name="wdp", bufs=8, side="left")  # wo (C-D)
        eTp = tc.alloc_tile_pool(name="eTp", bufs=1, side="right")
        att = tc.alloc_tile_pool(name="att", bufs=3, side="right")
        bcp = tc.alloc_tile_pool(name="bcp", bufs=3, side="right")
        psS = tc.alloc_tile_pool(name="psS", bufs=3, space="PSUM", side="right")
        psO = tc.alloc_tile_pool(name="psO", bufs=2, space="PSUM", side="right")

        aoT = aop.tile([128, DT, N], MM_DT, tag="aoT")
        wo_t = load_w_pool = None
        wo_t = []
        for dk in range(DT):
            wt = wdp.tile([128, D], MM_DT, tag="wo")
            nc.sync.dma_start(wt, wo_d[dk * 128:(dk + 1) * 128, :])
            wo_t.append(wt)

        for pair in range(H // 2):
            he, ho = 2 * pair, 2 * pair + 1
            for hf in range(2):
                ns = slice(hf * 512, (hf + 1) * 512)
                # scoresT for both heads of the pair, interleaved so the
                # even/odd matmuls run concurrently on distinct PE row groups
                eps_ = [
                    eTp.tile([128, 2, 2, 512], MM_DT, tag=f"eT{i}", name=f"ep{i}",
                             bufs=2 if DEEP else 1)
                    for i in range(4)
                ]
                for mb in range(NT):
                    pse = psS.tile([128, 1024], F32, tag="psS")
                    nc.tensor.matmul(
                        pse[:, 0:512],
                        kT[0:64, pair, mb * 128:(mb + 1) * 128],
                        qT[0:64, pair, ns],
                        start=True, stop=True,
                    )
                    nc.tensor.matmul(
                        pse[:, 512:1024],
                        kT[64:128, pair, mb * 128:(mb + 1) * 128],
                        qT[64:128, pair, ns],
                        start=True, stop=True,
                    )
                    nc.scalar.activation(
                        eps_[mb // 2][:, mb % 2], pse,
                        mybir.ActivationFunctionType.Exp, scale=SCALE,
                    )
                # attn @ v for both heads, interleaved accumulation chains
                pso_e = psO.tile([128, 512], F32, tag="psO")
                pso_o = psO.tile([128, 512], F32, tag="psO")
                for mb in range(NT):
                    e_mb = eps_[mb // 2][:, mb % 2]
                    nc.tensor.matmul(
                        pso_e[0:65, :], v_sb[:, mb, he, :], e_mb[:, 0, :],
                        start=(mb == 0), stop=(mb == NT - 1),
                    )
                    nc.tensor.matmul(
                        pso_o[0:65, :], v_sb[:, mb, ho, :], e_mb[:, 1, :],
                        start=(mb == 0), stop=(mb == NT - 1),
                    )
                # normalize: reciprocal denominator, DRAM-bounce broadcast
                for par, pso in ((0, pso_e), (1, pso_o)):
                    idx = 2 * (he + par) + hf
                    r_sb = att.tile([128, 512], F32, tag="r_sb")
                    nc.vector.tensor_copy(r_sb[0:65, :], pso[0:65, :])
                    nc.gpsimd.dma_start(
                        rb_d[idx:idx + 1, :], r_sb[64:65, :]
                    )
                    # reload the 512 sums across 128 partitions, reciprocal
                    # there (8 cyc/elem is per-lane), bounce back, broadcast
                    rt = att.tile([128, 4], F32, tag="rt")
                    nc.gpsimd.dma_start(
                        rt,
                        bass.AP(
                            tensor=rb_d.tensor,
                            offset=rb_d.offset + idx * 512,
                            ap=[[4, 128], [1, 4]],
                        ),
                    )
                    nc.vector.reciprocal(rt, rt)
                    nc.gpsimd.dma_start(
                        bass.AP(
                            tensor=rb_d.tensor,
                            offset=rb_d.offset + (32 + idx) * 512,
                            ap=[[4, 128], [1, 4]],
                        ),
                        rt,
                    )
                    bc = bcp.tile([64, 512], F32, tag="bc")
                    nc.gpsimd.dma_start(
                        bc,
                        bass.AP(
                            tensor=rb_d.tensor,
                            offset=rb_d.offset + (32 + idx) * 512,
                            ap=[[0, 64], [1, 512]],
                        ),
                    )
                    if par == 0:
                        nc.vector.tensor_mul(aoT[0:64, pair, ns], r_sb[0:64, :], bc)
                    else:
                        tmp = att.tile([64, 512], MM_DT, tag="tmp")
                        nc.vector.tensor_mul(tmp, r_sb[0:64, :], bc)
                        nc.gpsimd.dma_start(aoT[64:128, pair, ns], tmp)

        for p in (psO, psS, bcp, att, eTp, qkv):
            p.release()

        # ============ Phase D: out-proj + final LN ============
        finp = tc.alloc_tile_pool(name="finp", bufs=2, side="left")
        gbp = tc.alloc_tile_pool(name="gbp", bufs=1, side="left")
        psD = tc.alloc_tile_pool(name="psD", bufs=2, space="PSUM", side="left")

        g_bc = gbp.tile([128, D], F32, tag="g_bc")
        nc.sync.dma_start(g_bc, bcast_row(1))
        b_bc = gbp.tile([128, D], F32, tag="b_bc")
        nc.sync.dma_start(b_bc, bcast_row(2))

        for nt in range(NT):
            fin = finp.tile([128, D], F32, tag="fin")
            for hf in range(2):
                ds_ = slice(hf * 512, (hf + 1) * 512)
                ps = psD.tile([128, 512], F32, tag="psD")
                for dk in range(DT):
                    nc.tensor.matmul(
                        ps,
                        aoT[:, dk, nt * 128:(nt + 1) * 128],
                        wo_t[dk][:, ds_],
                        start=(dk == 0),
                        stop=(dk == DT - 1),
                    )
                nc.vector.tensor_copy(fin[:, ds_], ps)
            z = finp.tile([128, D], F32, tag="z")
            ln_tile(fin, z)
            nc.vector.tensor_mul(z, z, g_bc)
            nc.vector.tensor_add(z, z, b_bc)
            nc.sync.dma_start(out_d[nt * 128:(nt + 1) * 128, :], z)

        for p in (psD, gbp, finp, wdp, aop, mvp, stp, const):
            p.release()

    nc.compile()
    return nc


_NC_CACHE = None


def _get_nc():
    global _NC_CACHE
    if _NC_CACHE is None:
        _NC_CACHE = build_program()
    return _NC_CACHE


def _host_prep(inputs):
    f64 = np.float64
    Wq = inputs["Wq"].astype(f64)
    Wk = inputs["Wk"].astype(f64)
    Wv = inputs["Wv"].astype(f64)

    wq = (inputs["nq_g"].astype(f64)[:, None] * Wq).astype(np.float32)
    wk = (inputs["nk_g"].astype(f64)[:, None] * Wk).astype(np.float32)
    wv = (inputs["nv_g"].astype(f64)[:, None] * Wv).astype(np.float32)
    bq = (inputs["nq_b"].astype(f64) @ Wq + inputs["bq"].astype(f64)).astype(np.float32)
    bk = (inputs["nk_b"].astype(f64) @ Wk + inputs["bk"].astype(f64)).astype(np.float32)
    bv = (inputs["nv_b"].astype(f64) @ Wv + inputs["bv"].astype(f64)).astype(np.float32)

    # rope tables
    freqs = (1.0 / THETA ** (np.arange(0, DH, 2, dtype=np.float32) / DH)).astype(
        np.float32
    )
    t = np.arange(N, dtype=np.float32)
    ang = np.outer(t, freqs).astype(np.float64)  # [N, 32]
    cos_t = np.cos(ang).astype(np.float32)
    sin_t = np.sin(ang).astype(np.float32)
    p = np.arange(128)
    i_of_p = (p % 64) // 2
    cosf = np.ascontiguousarray(cos_t[:, i_of_p].T)  # [128, N]
    sgn = np.where(p % 2 == 0, -1.0, 1.0).astype(np.float32)
    sinf = np.ascontiguousarray(sin_t[:, i_of_p].T * sgn[:, None]).astype(np.float32)

    def btab(b):
        tab = np.zeros((128, DT), np.float32)
        tabs = np.zeros((128, DT), np.float32)
        for td in range(DT):
            tab[:, td] = b[td * 128 + p]
            tabs[:, td] = b[td * 128 + (p ^ 1)]
        return tab, tabs

    bqa, bqs = btab(bq)
    bka, bks = btab(bk)

    rows = np.stack(
        [bv, inputs["ln_g"].astype(np.float32), inputs["ln_b"].astype(np.float32)]
    )

    if MM_DT == BF16:
        import ml_dtypes
        wire = ml_dtypes.bfloat16
        wq, wk, wv = (a.astype(wire) for a in (wq, wk, wv))
        wo_w = np.ascontiguousarray(inputs["Wo"].astype(np.float64)).astype(wire)
    else:
        wo_w = np.ascontiguousarray(inputs["Wo"].astype(np.float32))
    return {
        "wq": wq, "wk": wk, "wv": wv,
        "wo": wo_w,
        "cosf": cosf, "sinf": sinf,
        "bqa": bqa, "bqs": bqs, "bka": bka, "bks": bks,
        "rows": rows.astype(np.float32),
    }


def run(inputs, trace=False, tmpdir=None):
    nc = _get_nc()
    shared = _host_prep(inputs)
    x = np.asarray(inputs["x"], np.float32)
    src = np.asarray(inputs["source"], np.float32)
    in_maps = [
        {"x": np.ascontiguousarray(x[c]), "src": np.ascontiguousarray(src[c]), **shared}
        for c in range(B)
    ]
    res = run_bass_kernel_spmd(nc, in_maps, list(range(B)), trace=trace, tmpdir=tmpdir)
    out = np.stack([res.results[c]["out"] for c in range(B)]).astype(np.float32)
    return out, res


def kernel(**inputs):
    return run(inputs)[0]
